# revision 1
# baseline (speedup 1.0000x reference)
"""2-layer GCN (PyG GCNConv x2 + leaky_relu) on 8 Trainium2 NeuronCores.

Strategy (dst-partitioned graph parallel):
  - Nodes partitioned across 8 cores (6250 dst nodes each). Each core owns the
    aggregation for its dst partition for both layers.
  - Normalization folded so the edge scatter matrix is pure 0/1:
      out[d] = dis[d] * sum_e (dis[s_e] * x[s_e]) + rank1-bias-term
    dis[s] folded into the gather table (host pre-scale for layer 1, ACT
    per-partition scale for layer 2's table build); dis[d] applied as ACT
    per-partition scale post-aggregation. Self-loops appended as edges.
  - Per 128-edge chunk: dma_gather of table rows (edge e -> partition e%128),
    one-hot S [128e,128d] built on DVE via tensor_scalar(iota is_equal dstloc),
    PE matmul psum[128d,64f] += S.T @ G accumulated per dst tile.
  - Layer1 per tile: agg -(ACT copy,*dis)-> sbuf -(PE transpose)-> aggT
    -(GEMM W1)-> x1T -(ACT Lrelu+b1)-> -(GEMM W2)-> xw2 -(ACT copy,*dis)->
    part. One AllGather of all parts builds the full layer-2 table [50000,64].
  - Layer2 per tile: same chunk matmuls from the gathered table, + a K=1
    rank-1 matmul adding outer(1/dis[d], b2) inside the PSUM group, then one
    ACT Lrelu with scale=dis[d].
  - b2 trick: Lrelu(dis*agg + b2) == Lrelu(dis*(agg + outer(1/dis, b2))).

Self-contained: hardcodes shapes from the problem spec; builds/compiles on
first call keyed by input hash.
"""

import os
import sys
import hashlib

import numpy as np

sys.path.insert(0, "/opt/trn_rl_repo")

# ---- problem constants (from spec) ----
N, E = 50000, 800000
DIN, DH, DOUT = 64, 128, 64
P_CORES = 8
NP = N // P_CORES          # 6250 dst nodes per core
TILE = 128                 # dst tile width
NT = (NP + TILE - 1) // TILE   # 49 tiles (48*128 + 106)
LAST_ROWS = NP - (NT - 1) * TILE  # 106
SG = 7                     # dst tiles per gather supergroup
NSG = (NT + SG - 1) // SG  # 7 supergroups
H = N // 2                 # int16 half split of the gather tables
PAD_DST = 200.0            # dstloc value for pad edges (no iota match)
NEG_SLOPE = 0.01


def _prep(edge_index: np.ndarray):
    """Bucket edges (incl. self loops) by (core, dst_tile, src_half); pad each
    group to a multiple of 128; emit per-core SBUF-layout idx/dstloc arrays and
    the shared per-group chunk counts (max over cores -> single SPMD program).
    """
    src = np.asarray(edge_index[0], dtype=np.int64)
    dst = np.asarray(edge_index[1], dtype=np.int64)
    loops = np.arange(N, dtype=np.int64)
    s_all = np.concatenate([src, loops])
    d_all = np.concatenate([dst, loops])

    deg = (np.bincount(dst, minlength=N) + 1).astype(np.float32)
    dis = (1.0 / np.sqrt(deg)).astype(np.float32)

    core = d_all // NP
    dl = d_all - core * NP            # local dst 0..NP-1
    tl = dl // TILE                   # dst tile 0..NT-1
    half = (s_all >= H).astype(np.int64)

    # group id = core*NT*2 + tile*2 + half
    gid = core * (NT * 2) + tl * 2 + half
    order = np.argsort(gid, kind="stable")
    gid_s = gid[order]
    s_s = s_all[order]
    dloc_s = (dl - tl * TILE)[order]

    n_groups = P_CORES * NT * 2
    cnt = np.bincount(gid_s, minlength=n_groups).reshape(P_CORES, NT, 2)
    starts = np.zeros(n_groups + 1, dtype=np.int64)
    np.cumsum(cnt.reshape(-1), out=starts[1:])

    # chunks per (tile, half): max over cores, >=1
    nch = np.maximum(1, (cnt.max(axis=0) + 127) // 128)  # [NT, 2]

    # chunk order inside a layer: for sg in NSG: tiles t in sg: A chunks of
    # all sg tiles, then B chunks of all sg tiles.
    # col offset of each (tile, half) group in the global chunk stream:
    chunk_off = np.zeros((NT, 2), dtype=np.int64)
    sg_info = []  # per sg: (t0, tcnt, colA0, nA, colB0, nB)
    col = 0
    for g in range(NSG):
        t0 = g * SG
        tcnt = min(SG, NT - t0)
        colA0 = col
        for t in range(t0, t0 + tcnt):
            chunk_off[t, 0] = col
            col += nch[t, 0]
        nA = col - colA0
        colB0 = col
        for t in range(t0, t0 + tcnt):
            chunk_off[t, 1] = col
            col += nch[t, 1]
        nB = col - colB0
        sg_info.append((t0, tcnt, colA0, int(nA), colB0, int(nB)))
    total_chunks = int(col)
    max_sg_chunks = max(nA + nB for (_, _, _, nA, _, nB) in sg_info)

    # per-core flat streams
    idx_flat = np.zeros((P_CORES, total_chunks * 128), dtype=np.int16)
    dst_flat = np.full((P_CORES, total_chunks * 128), PAD_DST, dtype=np.float32)
    for c in range(P_CORES):
        for t in range(NT):
            for h in range(2):
                g = c * (NT * 2) + t * 2 + h
                a, b = starts[g], starts[g + 1]
                n = b - a
                o = chunk_off[t, h] * 128
                iv = s_s[a:b] - (H if h else 0)
                idx_flat[c, o:o + n] = iv.astype(np.int16)
                dst_flat[c, o:o + n] = dloc_s[a:b].astype(np.float32)

    # SBUF layouts
    # idx: per gather call, element i -> partition i%16, col i//16; replicate
    # the 16-row pattern to all 128 partitions. Calls are (sg, half) segments
    # of whole chunks, so per-chunk wrapping == per-call wrapping.
    idx_sb = np.zeros((P_CORES, 128, total_chunks * 8), dtype=np.int16)
    for c in range(P_CORES):
        w = idx_flat[c].reshape(total_chunks * 8, 16).T  # [16, cols]
        idx_sb[c] = np.tile(w, (8, 1))
    dst_sb = np.transpose(
        dst_flat.reshape(P_CORES, total_chunks, 128), (0, 2, 1)
    ).copy()  # [P, 128, total_chunks]

    # per-core dis columns for its dst tiles, padded to NT*128
    dis_pad = np.zeros((P_CORES, NT * TILE), dtype=np.float32)
    disinv_pad = np.zeros((P_CORES, NT * TILE), dtype=np.float32)
    for c in range(P_CORES):
        d = dis[c * NP:(c + 1) * NP]
        dis_pad[c, :NP] = d
        disinv_pad[c, :NP] = 1.0 / d
    dis_sb = dis_pad.reshape(P_CORES, NT, TILE).transpose(0, 2, 1).copy()  # [P,128,NT]

    return dict(
        dis=dis,
        nch=nch,
        chunk_off=chunk_off,
        sg_info=sg_info,
        total_chunks=total_chunks,
        max_sg_chunks=max_sg_chunks,
        idx_sb=idx_sb,
        dst_sb=dst_sb,
        dis_sb=dis_sb,
        disinv=disinv_pad,  # [P, NT*128]
    )


def _numpy_pipeline(inputs, edge_index, W1, b1, W2, b2, prep, leaky=True):
    """Host model of the exact device pipeline (for validation)."""
    dis = prep["dis"]
    nch = prep["nch"]
    chunk_off = prep["chunk_off"]
    xt = (inputs * dis[:, None]).astype(np.float32)  # layer-1 table

    def slope(x):
        return np.where(x > 0, x, NEG_SLOPE * x) if leaky else np.maximum(x, 0)

    table = np.zeros((N, DOUT), dtype=np.float32)  # layer-2 table (xw2*dis)
    out = np.zeros((N, DOUT), dtype=np.float32)

    for layer in range(2):
        tab = xt if layer == 0 else table
        for c in range(P_CORES):
            idx_flat = prep["idx_sb"][c][:16].T.reshape(-1)  # unwrap
            dst_flat = prep["dst_sb"][c].T.reshape(-1)
            for t in range(NT):
                agg = np.zeros((TILE, tab.shape[1]), dtype=np.float32)
                for h in range(2):
                    o = chunk_off[t, h]
                    for k in range(nch[t, h]):
                        cc = o + k
                        ii = idx_flat[cc * 128:(cc + 1) * 128].astype(np.int64)
                        dd = dst_flat[cc * 128:(cc + 1) * 128]
                        base = H if h else 0
                        G = tab[base + ii]  # [128, F]
                        S = (np.arange(TILE)[None, :] == dd[:, None]).astype(np.float32)
                        agg += S.T @ G
                dis_col = prep["dis_sb"][c][:, t]  # [128]
                rows = LAST_ROWS if t == NT - 1 else TILE
                n0 = c * NP + t * TILE
                if layer == 0:
                    aggd = agg * dis_col[:, None]
                    x1 = slope(aggd @ W1 + b1[None, :])
                    xw2 = (x1 @ W2) * dis_col[:, None]
                    table[n0:n0 + rows] = xw2[:rows]
                else:
                    dinv = prep["disinv"][c][t * TILE:(t + 1) * TILE]
                    agg2 = agg + np.outer(dinv, b2)
                    out[n0:n0 + rows] = slope(agg2 * dis_col[:, None])[:rows]
    return out


# ---------------------------------------------------------------------------
# Bass kernel
# ---------------------------------------------------------------------------

def _build_nc(prep, use_lrelu=True, bf16=False):
    import concourse.bass as bass
    import concourse.bacc as bacc
    import concourse.tile as tile
    from concourse import mybir

    f32 = mybir.dt.float32
    bfl = mybir.dt.bfloat16
    gdt = bfl if bf16 else f32        # gather-table / S / G dtype
    TW = 128 if bf16 else DIN         # table row width (256B either way)
    i16 = mybir.dt.int16
    AF = mybir.ActivationFunctionType
    ALU = mybir.AluOpType

    nch = prep["nch"]
    chunk_off = prep["chunk_off"]
    sg_info = prep["sg_info"]
    TC = prep["total_chunks"]
    NCHMAX = prep["max_sg_chunks"]

    nc = bacc.Bacc(
        "TRN2", target_bir_lowering=False, debug=False,
        enable_asserts=False, num_devices=P_CORES,
    )

    # I/O
    xt_d = nc.dram_tensor("xt", [N, TW], gdt, kind="ExternalInput")
    idx_d = nc.dram_tensor("idx", [128, TC * 8], i16, kind="ExternalInput")
    dstloc_d = nc.dram_tensor("dstloc", [128, TC], gdt, kind="ExternalInput")
    dis_d = nc.dram_tensor("dis_t", [128, NT], f32, kind="ExternalInput")
    disinv_d = nc.dram_tensor("disinv", [1, NT * TILE], f32, kind="ExternalInput")
    w1_d = nc.dram_tensor("w1", [DIN, DH], f32, kind="ExternalInput")
    w2_d = nc.dram_tensor("w2", [DH, DOUT], f32, kind="ExternalInput")
    b1_d = nc.dram_tensor("b1", [DH, 1], f32, kind="ExternalInput")
    b2_d = nc.dram_tensor("b2r", [1, DOUT], f32, kind="ExternalInput")
    iota_d = nc.dram_tensor("iota", [128, 128], gdt, kind="ExternalInput")
    ident_d = nc.dram_tensor("ident", [128, 128], f32, kind="ExternalInput")
    out_d = nc.dram_tensor("outp", [NP, DOUT], f32, kind="ExternalOutput")

    with tile.TileContext(nc) as tc:
        with (
            tc.tile_pool(name="const", bufs=1) as constp,
            tc.tile_pool(name="gbuf", bufs=2) as gpool,
            tc.tile_pool(name="spool", bufs=6) as spool,
            tc.tile_pool(name="work", bufs=2) as work,
            tc.tile_pool(name="pagg", bufs=2, space="PSUM") as pagg,
            tc.tile_pool(name="ptr", bufs=2, space="PSUM") as ptr,
            tc.tile_pool(name="pg1", bufs=2, space="PSUM") as pg1,
            tc.tile_pool(name="pg2", bufs=2, space="PSUM") as pg2,
            tc.tile_pool(name="dram", bufs=1, space="DRAM") as dram,
        ):
            # constants / streams
            idx_sb = constp.tile([128, TC * 8], i16)
            dstloc_sb = constp.tile([128, TC], gdt)
            dis_sb = constp.tile([128, NT], f32)
            disinv_sb = constp.tile([1, NT * TILE], f32)
            w1_sb = constp.tile([DIN, DH], f32)
            w2_sb = constp.tile([DH, DOUT], f32)
            b1_sb = constp.tile([DH, 1], f32)
            b2_sb = constp.tile([1, DOUT], f32)
            iota_sb = constp.tile([128, 128], gdt)
            ident_sb = constp.tile([128, 128], f32)
            for sb, d in [
                (idx_sb, idx_d), (dstloc_sb, dstloc_d), (dis_sb, dis_d),
                (disinv_sb, disinv_d), (w1_sb, w1_d), (w2_sb, w2_d),
                (b1_sb, b1_d), (b2_sb, b2_d), (iota_sb, iota_d),
                (ident_sb, ident_d),
            ]:
                nc.sync.dma_start(sb[:], d[:])

            part = None   # allocated per repeat-iteration below
            table = None

            def layer(lidx, in_ap_A, in_ap_B, felems):
                """Emit one GCN layer. felems = table row width."""
                for (t0, tcnt, colA0, nA, colB0, nB) in sg_info:
                    g_t = gpool.tile([128, NCHMAX, felems], gdt, tag="gbuf")
                    CAP = 48  # chunks per dma_gather call (HW SWDGE ring limit)
                    for (c0, ncnk, in_ap) in (
                        (colA0, nA, in_ap_A), (colB0, nB, in_ap_B),
                    ):
                        if os.environ.get("GCN_ABLATE", "") == "nogather":
                            continue
                        for off in range(0, ncnk, CAP):
                            n = min(CAP, ncnk - off)
                            num = n * 128
                            cc0 = c0 + off
                            sgc0 = cc0 - colA0  # column inside g_t
                            nc.gpsimd.dma_gather(
                                out_ap=g_t[:, sgc0:sgc0 + n, :],
                                in_ap=in_ap,
                                idxs_ap=idx_sb[:, cc0 * 8:(cc0 + n) * 8],
                                num_idxs=num,
                                num_idxs_reg=num,
                                elem_size=felems,
                                single_packet=False,
                            )
                    ablate = os.environ.get("GCN_ABLATE", "")
                    for t in range(t0, t0 + tcnt):
                        rows = LAST_ROWS if t == NT - 1 else TILE
                        agg_p = pagg.tile([TILE, DIN], f32, tag="pagg")
                        ids = [
                            chunk_off[t, h] + k
                            for h in range(2) for k in range(nch[t, h])
                        ]
                        if ablate == "nosmm":
                            ids = ids[:1]
                        for j, cc in enumerate(ids):
                            s_t = spool.tile([128, TILE], gdt, tag="spool")
                            nc.vector.tensor_scalar(
                                s_t[:], iota_sb[:],
                                dstloc_sb[:, cc:cc + 1], None, ALU.is_equal,
                            )
                            last = (j == len(ids) - 1) and lidx == 0
                            nc.tensor.matmul(
                                agg_p[:], s_t[:],
                                g_t[:, cc - colA0, 0:DIN],
                                start=(j == 0), stop=last,
                            )
                        if lidx == 0:
                            # copy+dis -> transpose -> GEMM1 -> lrelu -> GEMM2
                            agg_sb = work.tile([TILE, DIN], f32, tag="agg_sb")
                            nc.scalar.activation(
                                agg_sb[:], agg_p[:], AF.Copy,
                                bias=0.0, scale=dis_sb[:, t:t + 1],
                            )
                            aggT_p = ptr.tile([DIN, TILE], f32, tag="ptr")
                            nc.tensor.transpose(aggT_p[:], agg_sb[:], ident_sb[:])
                            aggT_sb = work.tile([DIN, TILE], f32, tag="aggT_sb")
                            nc.scalar.activation(
                                aggT_sb[:], aggT_p[:], AF.Copy, bias=0.0,
                            )
                            x1T_p = pg1.tile([DH, TILE], f32, tag="pg1")
                            nc.tensor.matmul(
                                x1T_p[:], w1_sb[:], aggT_sb[:],
                                start=True, stop=True,
                            )
                            x1T_sb = work.tile([DH, TILE], f32, tag="x1T_sb")
                            nc.scalar.activation(
                                x1T_sb[:], x1T_p[:],
                                AF.Lrelu if use_lrelu else AF.Relu,
                                bias=b1_sb[:, 0:1], alpha=NEG_SLOPE,
                            )
                            xw2_p = pg2.tile([TILE, DOUT], f32, tag="pg2")
                            nc.tensor.matmul(
                                xw2_p[:], x1T_sb[:], w2_sb[:],
                                start=True, stop=True,
                            )
                            xw2_sb = work.tile([TILE, DOUT], gdt, tag="xw2_sb")
                            nc.scalar.activation(
                                xw2_sb[:], xw2_p[:], AF.Copy,
                                bias=0.0, scale=dis_sb[:, t:t + 1],
                            )
                            nc.sync.dma_start(
                                part[t * TILE:t * TILE + rows, 0:DOUT],
                                xw2_sb[:rows, :],
                            )
                        else:
                            # rank-1 b2 term closes the PSUM group
                            nc.tensor.matmul(
                                agg_p[:],
                                disinv_sb[:, t * TILE:(t + 1) * TILE],
                                b2_sb[:],
                                start=False, stop=True,
                            )
                            o_sb = work.tile([TILE, DOUT], f32, tag="o_sb")
                            nc.scalar.activation(
                                o_sb[:], agg_p[:],
                                AF.Lrelu if use_lrelu else AF.Relu,
                                bias=0.0, scale=dis_sb[:, t:t + 1],
                                alpha=NEG_SLOPE,
                            )
                            nc.sync.dma_start(
                                out_d[t * TILE:t * TILE + rows, :],
                                o_sb[:rows, :],
                            )

            for _rep in range(int(os.environ.get("GCN_REPEAT", "1"))):
                part = dram.tile([NP, TW], gdt, tag="part", bufs=2)
                table = dram.tile([N, TW], gdt, addr_space="Shared",
                                  tag="table", bufs=2)
                # layer 1: gather from xt (ExternalInput DRAM)
                layer(0, xt_d[0:H, :], xt_d[H:N, :], TW)
                # exchange
                nc.gpsimd.collective_compute(
                    "AllGather", mybir.AluOpType.bypass,
                    replica_groups=[list(range(P_CORES))],
                    ins=[part.opt()], outs=[table.opt()],
                )
                # layer 2: gather from table
                layer(1, table[0:H, :], table[H:N, :], TW)

    nc.compile()
    return nc



# ---------------------------------------------------------------------------
# Looped (small-program) variant: uniform chunk counts, For_i over dst tiles.
# ---------------------------------------------------------------------------

NPP = NT * TILE          # padded rows per core (6272)
H2 = (P_CORES // 2) * NPP  # half split of the padded table (25088)


def _prep_loop(edge_index: np.ndarray):
    src = np.asarray(edge_index[0], dtype=np.int64)
    dst = np.asarray(edge_index[1], dtype=np.int64)
    loops = np.arange(N, dtype=np.int64)
    s_all = np.concatenate([src, loops])
    d_all = np.concatenate([dst, loops])

    deg = (np.bincount(dst, minlength=N) + 1).astype(np.float32)
    dis = (1.0 / np.sqrt(deg)).astype(np.float32)

    srow = (s_all // NP) * NPP + (s_all % NP)   # padded table row of src
    half = (srow >= H2).astype(np.int64)
    idxv = srow - half * H2                      # int16-safe (< 25088)

    core = d_all // NP
    dl = d_all - core * NP
    tl = dl // TILE
    dloc = dl - tl * TILE

    gid = ((core * NT) + tl) * 2 + half
    order = np.argsort(gid, kind="stable")
    gid_s, idx_s, dloc_s = gid[order], idxv[order], dloc[order]
    ngroups = P_CORES * NT * 2
    cnt = np.bincount(gid_s, minlength=ngroups).reshape(P_CORES, NT, 2)
    starts = np.zeros(ngroups + 1, dtype=np.int64)
    np.cumsum(cnt.reshape(-1), out=starts[1:])

    ncha = int((cnt[:, :, 0].max() + 127) // 128)
    nchb = int((cnt[:, :, 1].max() + 127) // 128)
    ncht = ncha + nchb
    TC = NT * ncht

    idx_flat = np.zeros((P_CORES, TC * 128), dtype=np.int16)
    dst_flat = np.full((P_CORES, TC * 128), PAD_DST, dtype=np.float32)
    for c in range(P_CORES):
        for t in range(NT):
            for h in range(2):
                g = ((c * NT) + t) * 2 + h
                a, b = starts[g], starts[g + 1]
                o = (t * ncht + (0 if h == 0 else ncha)) * 128
                idx_flat[c, o:o + b - a] = idx_s[a:b].astype(np.int16)
                dst_flat[c, o:o + b - a] = dloc_s[a:b].astype(np.float32)

    idx_sb = np.zeros((P_CORES, 128, TC * 8), dtype=np.int16)
    for c in range(P_CORES):
        w = idx_flat[c].reshape(TC * 8, 16).T
        idx_sb[c] = np.tile(w, (8, 1))
    dst_sb = np.transpose(dst_flat.reshape(P_CORES, TC, 128), (0, 2, 1)).copy()

    dis_pad = np.zeros((P_CORES, NPP), dtype=np.float32)
    dinv_pad = np.zeros((P_CORES, NPP), dtype=np.float32)
    for c in range(P_CORES):
        d = dis[c * NP:(c + 1) * NP]
        dis_pad[c, :NP] = d
        dinv_pad[c, :NP] = 1.0 / d
    dis_sb = dis_pad.reshape(P_CORES, NT, TILE).transpose(0, 2, 1).copy()

    return dict(
        dis=dis, ncha=ncha, nchb=nchb, ncht=ncht, TC=TC,
        idx_sb=idx_sb, dst_sb=dst_sb, dis_sb=dis_sb, disinv=dinv_pad,
    )


def _build_nc_loop(prep):
    import concourse.bass as bass
    import concourse.bacc as bacc
    import concourse.tile as tile
    from concourse import mybir

    f32 = mybir.dt.float32
    i16 = mybir.dt.int16
    AF = mybir.ActivationFunctionType
    ALU = mybir.AluOpType
    ds = bass.ds

    NCHA, NCHB = prep["ncha"], prep["nchb"]
    NCHT = prep["ncht"]
    TC = prep["TC"]

    nc = bacc.Bacc(
        "TRN2", target_bir_lowering=False, debug=False,
        enable_asserts=False, num_devices=P_CORES,
    )

    xt_d = nc.dram_tensor("xt", [P_CORES * NPP, DIN], f32, kind="ExternalInput")
    idx_d = nc.dram_tensor("idx", [128, TC * 8], i16, kind="ExternalInput")
    dstloc_d = nc.dram_tensor("dstloc", [128, TC], f32, kind="ExternalInput")
    dis_d = nc.dram_tensor("dis_t", [128, NT], f32, kind="ExternalInput")
    disinv_d = nc.dram_tensor("disinv", [1, NPP], f32, kind="ExternalInput")
    w1_d = nc.dram_tensor("w1", [DIN, DH], f32, kind="ExternalInput")
    w2_d = nc.dram_tensor("w2", [DH, DOUT], f32, kind="ExternalInput")
    b1_d = nc.dram_tensor("b1", [DH, 1], f32, kind="ExternalInput")
    b2_d = nc.dram_tensor("b2r", [1, DOUT], f32, kind="ExternalInput")
    b2rep_d = nc.dram_tensor("b2rep", [128, DOUT], f32, kind="ExternalInput")
    iota_d = nc.dram_tensor("iota", [128, 128], f32, kind="ExternalInput")
    ident_d = nc.dram_tensor("ident", [128, 128], f32, kind="ExternalInput")
    out_d = nc.dram_tensor("outp", [NPP, DOUT], f32, kind="ExternalOutput")

    with tile.TileContext(nc) as tc:
        with (
            tc.tile_pool(name="const", bufs=1) as constp,
            tc.tile_pool(name="gbuf", bufs=2) as gpool,
            tc.tile_pool(name="spool", bufs=4) as spool,
            tc.tile_pool(name="work", bufs=2) as work,
            tc.tile_pool(name="pagg", bufs=2, space="PSUM") as pagg,
            tc.tile_pool(name="ptr", bufs=2, space="PSUM") as ptr,
            tc.tile_pool(name="pg1", bufs=2, space="PSUM") as pg1,
            tc.tile_pool(name="pg2", bufs=2, space="PSUM") as pg2,
            tc.tile_pool(name="dram", bufs=1, space="DRAM") as dram,
        ):
            idx_sb = constp.tile([128, TC * 8], i16)
            dstloc_sb = constp.tile([128, TC], f32)
            dis_sb = constp.tile([128, NT], f32)
            disinv_sb = constp.tile([1, NPP], f32)
            w1_sb = constp.tile([DIN, DH], f32)
            w2_sb = constp.tile([DH, DOUT], f32)
            b1_sb = constp.tile([DH, 1], f32)
            b2_sb = constp.tile([1, DOUT], f32)
            b2rep_sb = constp.tile([128, DOUT], f32)
            iota_sb = constp.tile([128, 128], f32)
            ident_sb = constp.tile([128, 128], f32)
            for sb, dr in [
                (idx_sb, idx_d), (dstloc_sb, dstloc_d), (dis_sb, dis_d),
                (disinv_sb, disinv_d), (w1_sb, w1_d), (w2_sb, w2_d),
                (b1_sb, b1_d), (b2_sb, b2_d), (b2rep_sb, b2rep_d),
                (iota_sb, iota_d), (ident_sb, ident_d),
            ]:
                nc.sync.dma_start(sb[:], dr[:])

            for _rep in range(int(os.environ.get("GCN_REPEAT", "1"))):
                part = dram.tile([NPP, DOUT], f32, tag="part", bufs=2)
                table = dram.tile([P_CORES * NPP, DOUT], f32,
                                  addr_space="Shared", tag="table", bufs=2)

                for lidx in range(2):
                    src_t = xt_d if lidx == 0 else table
                    in_A = src_t[0:H2, :]
                    in_B = src_t[H2:2 * H2, :]
                    with tc.For_i(0, NT, 1) as i:
                        g_t = gpool.tile([128, NCHT, DIN], f32, tag="gbuf")
                        stage = gpool.tile([128, NCHT * 8], i16,
                                           tag="stage", bufs=2)
                        nc.sync.dma_start(
                            stage[:], idx_d[:, ds(i * (NCHT * 8), NCHT * 8)])
                        dstage = gpool.tile([128, NCHT], f32,
                                            tag="dstage", bufs=2)
                        nc.sync.dma_start(
                            dstage[:], dstloc_d[:, ds(i * NCHT, NCHT)])
                        sstage = gpool.tile([128, 1], f32,
                                            tag="sstage", bufs=2)
                        nc.sync.dma_start(
                            sstage[:], dis_d[:, ds(i, 1)])
                        for (c0, ncnk, in_ap) in (
                            (0, NCHA, in_A), (NCHA, NCHB, in_B),
                        ):
                            nc.gpsimd.dma_gather(
                                out_ap=g_t[:, c0:c0 + ncnk, :],
                                in_ap=in_ap,
                                idxs_ap=stage[:, c0 * 8:(c0 + ncnk) * 8],
                                num_idxs=ncnk * 128,
                                num_idxs_reg=ncnk * 128,
                                elem_size=DIN,
                                single_packet=False,
                            )
                        agg_p = pagg.tile([TILE, DIN], f32, tag="pagg")
                        for j in range(NCHT):
                            s_t = spool.tile([128, TILE], f32, tag="spool")
                            nc.vector.tensor_scalar(
                                s_t[:], iota_sb[:],
                                dstage[:, j:j + 1], None,
                                ALU.is_equal,
                            )
                            nc.tensor.matmul(
                                agg_p[:], s_t[:], g_t[:, j, :],
                                start=(j == 0),
                                stop=(j == NCHT - 1),
                            )
                        if lidx == 0:
                            agg_sb = work.tile([TILE, DIN], f32, tag="agg_sb")
                            nc.scalar.activation(
                                agg_sb[:], agg_p[:], AF.Copy,
                                bias=0.0, scale=sstage[:, 0:1],
                            )
                            aggT_p = ptr.tile([DIN, TILE], f32, tag="ptr")
                            nc.tensor.transpose(aggT_p[:], agg_sb[:], ident_sb[:])
                            aggT_sb = work.tile([DIN, TILE], f32, tag="aggT_sb")
                            nc.scalar.activation(
                                aggT_sb[:], aggT_p[:], AF.Copy, bias=0.0)
                            x1T_p = pg1.tile([DH, TILE], f32, tag="pg1")
                            nc.tensor.matmul(
                                x1T_p[:], w1_sb[:], aggT_sb[:],
                                start=True, stop=True)
                            x1T_sb = work.tile([DH, TILE], f32, tag="x1T_sb")
                            nc.scalar.activation(
                                x1T_sb[:], x1T_p[:], AF.Lrelu,
                                bias=b1_sb[:, 0:1], alpha=NEG_SLOPE)
                            xw2_p = pg2.tile([TILE, DOUT], f32, tag="pg2")
                            nc.tensor.matmul(
                                xw2_p[:], x1T_sb[:], w2_sb[:],
                                start=True, stop=True)
                            xw2_sb = work.tile([TILE, DOUT], f32, tag="xw2_sb")
                            nc.scalar.activation(
                                xw2_sb[:], xw2_p[:], AF.Copy,
                                bias=0.0, scale=sstage[:, 0:1])
                            nc.sync.dma_start(
                                part[ds(i * TILE, TILE), :], xw2_sb[:])
                        else:
                            t_sb = work.tile([TILE, DOUT], f32, tag="t_sb")
                            nc.vector.tensor_scalar(
                                t_sb[:], agg_p[:],
                                sstage[:, 0:1], None, ALU.mult)
                            u_sb = work.tile([TILE, DOUT], f32, tag="u_sb")
                            nc.vector.tensor_tensor(
                                u_sb[:], t_sb[:], b2rep_sb[:], op=ALU.add)
                            o_sb = work.tile([TILE, DOUT], f32, tag="o_sb")
                            nc.scalar.activation(
                                o_sb[:], u_sb[:], AF.Lrelu,
                                bias=0.0, alpha=NEG_SLOPE)
                            nc.sync.dma_start(
                                out_d[ds(i * TILE, TILE), :], o_sb[:])
                    if lidx == 0:
                        nc.gpsimd.collective_compute(
                            "AllGather", mybir.AluOpType.bypass,
                            replica_groups=[list(range(P_CORES))],
                            ins=[part.opt()], outs=[table.opt()],
                        )

    nc.compile()
    return nc


def _make_in_maps_loop(inputs, W1, b1, W2, b2, prep):
    xt32 = (np.asarray(inputs, np.float32) * prep["dis"][:, None]).astype(np.float32)
    xt = np.zeros((P_CORES * NPP, DIN), dtype=np.float32)
    for c in range(P_CORES):
        xt[c * NPP:c * NPP + NP] = xt32[c * NP:(c + 1) * NP]
    iota = np.tile(np.arange(128, dtype=np.float32), (128, 1))
    ident = np.eye(128, dtype=np.float32)
    in_maps = []
    for c in range(P_CORES):
        in_maps.append({
            "xt": xt,
            "idx": prep["idx_sb"][c],
            "dstloc": prep["dst_sb"][c],
            "dis_t": prep["dis_sb"][c],
            "disinv": prep["disinv"][c][None, :],
            "w1": np.asarray(W1, np.float32),
            "w2": np.asarray(W2, np.float32),
            "b1": np.asarray(b1, np.float32).reshape(DH, 1),
            "b2r": np.asarray(b2, np.float32).reshape(1, DOUT),
            "b2rep": np.tile(np.asarray(b2, np.float32), (128, 1)),
            "iota": iota,
            "ident": ident,
        })
    return in_maps


def _make_in_maps(inputs, W1, b1, W2, b2, prep, bf16=False):
    xt32 = (np.asarray(inputs, np.float32) * prep["dis"][:, None]).astype(np.float32)
    if bf16:
        import ml_dtypes
        bdt = ml_dtypes.bfloat16
        xt = np.zeros((N, 128), dtype=bdt)
        xt[:, :DIN] = xt32.astype(bdt)
    else:
        xt = xt32
    iota = np.tile(np.arange(128, dtype=np.float32), (128, 1))
    ident = np.eye(128, dtype=np.float32)
    in_maps = []
    for c in range(P_CORES):
        in_maps.append({
            "xt": xt,
            "idx": prep["idx_sb"][c],
            "dstloc": prep["dst_sb"][c].astype(bdt) if bf16 else prep["dst_sb"][c],
            "dis_t": prep["dis_sb"][c],
            "disinv": prep["disinv"][c][None, :],
            "w1": np.asarray(W1, np.float32),
            "w2": np.asarray(W2, np.float32),
            "b1": np.asarray(b1, np.float32).reshape(DH, 1),
            "b2r": np.asarray(b2, np.float32).reshape(1, DOUT),
            "iota": iota.astype(bdt) if bf16 else iota,
            "ident": ident,
        })
    return in_maps


_CACHE = {}


def _use_bf16():
    return os.environ.get("GCN_DTYPE", "f32") == "bf16"


def _use_loop():
    return os.environ.get("GCN_LOOP", "0") == "1"


def _get_compiled(edge_key, edge_index):
    if edge_key not in _CACHE:
        if _use_loop():
            prep = _prep_loop(edge_index)
            nc = _build_nc_loop(prep)
        else:
            prep = _prep(edge_index)
            nc = _build_nc(prep, bf16=_use_bf16())
        _CACHE[edge_key] = (prep, nc)
    return _CACHE[edge_key]


def kernel(inputs, edge_index, W1, b1, W2, b2, _trace=False, _results_box=None):
    from concourse.bass_utils import run_bass_kernel_spmd

    edge_index = np.asarray(edge_index)
    key = hashlib.sha1(edge_index.tobytes()).hexdigest()
    key += ":r%s:%s:%s:%s" % (os.environ.get("GCN_REPEAT", "1"),
                        os.environ.get("GCN_DTYPE", "f32"),
                        os.environ.get("GCN_ABLATE", ""),
                        os.environ.get("GCN_LOOP", "1"))
    prep, nc = _get_compiled(key, edge_index)
    if _use_loop():
        in_maps = _make_in_maps_loop(inputs, W1, b1, W2, b2, prep)
    else:
        in_maps = _make_in_maps(inputs, W1, b1, W2, b2, prep, bf16=_use_bf16())
    res = run_bass_kernel_spmd(
        nc, in_maps, core_ids=list(range(P_CORES)), trace=_trace,
    )
    if _results_box is not None:
        _results_box.append(res)
    if _use_loop():
        out = np.concatenate(
            [res.results[c]["outp"][:NP] for c in range(P_CORES)], axis=0
        )
    else:
        out = np.concatenate(
            [res.results[c]["outp"] for c in range(P_CORES)], axis=0
        )
    return out.astype(np.float32)



# revision 18
# speedup vs baseline: 4.9433x; 4.9433x over previous
"""2-layer GCN (PyG GCNConv x2 + leaky_relu) on 8 Trainium2 NeuronCores.

v2 strategy (dst-partitioned, gather-free, For_i hardware loops):
  - Nodes partitioned 128-ALIGNED across 8 cores: core c owns padded ids
    [c*6272, c*6272+6250); padded table has NPAD=50176 rows (zeros in pads).
  - Normalization folded: table rows pre-scaled by dis[src]; dis[dst] applied
    post-aggregation (ACT scale). Self-loops NOT in the edge stream: handled
    by one identity matmul per dst tile reading the core's own rows
    (xown for layer 1, `part` for layer 2) — contributes dis_d^2 * x_d.
  - Edge routing has NO per-edge DMA. Two phases through a DRAM scratch M:
    Phase 1 (For_i over 392 src blocks): one-hot Sel (DVE is_equal vs iota)
      selects/duplicates rows of X_b [128,64] into bucket slots via PE
      matmul; 7 chunks/block -> stage [128,7*64] -> one contiguous DMA to
      M block region [896 rows, 64]. Block region row m holds slot
      (p=m//7, c=m%7); bucket (b,t) occupies rows m = t*16 + r (r<16).
    Phase 2 (For_i over 49 dst tiles): one strided DMA reads rows
      [16t,16t+16) of every block -> msb [128, 49*64] (slot q=b*16+r at
      partition q//49, col q%49); 49 one-hot scatter matmuls accumulate
      agg[128dst,64] in PSUM; + identity matmul (self loop) (+ rank-1
      disinv x b2 term closing layer 2's group).
  - Layer-1 tail per tile: ACT(dis) -> PE transpose -> W1 -> Lrelu+b1 ->
    W2 -> ACT(dis) -> part. One AllGather builds the layer-2 table.
  - Bucket capacity R=16 (R grows by 16s if the graph needs it).

Self-contained: hardcodes shapes; compiles on first call keyed by edge hash.
"""

import os
import hashlib
import sys

import numpy as np

sys.path.insert(0, "/opt/trn_rl_repo")

# ---- problem constants ----
N, E = 50000, 800000
DIN, DH, DOUT = 64, 128, 64
P_CORES = 8
NP = N // P_CORES            # 6250 real nodes per core
NT = 49                      # dst tiles per core
NPP = NT * 128               # 6272 padded rows per core
NPAD = P_CORES * NPP         # 50176 padded table rows
NB = NPAD // 128             # 392 real src blocks (global)
NBM = 512                    # M-scratch block slots (pad => CPT = 4R exactly)
PAD = 200.0                  # one-hot miss value
NEG_SLOPE = 0.01


def _prep(edge_index: np.ndarray):
    src = np.asarray(edge_index[0], dtype=np.int64)
    dst = np.asarray(edge_index[1], dtype=np.int64)

    deg = (np.bincount(dst, minlength=N) + 1).astype(np.float32)
    dis = (1.0 / np.sqrt(deg)).astype(np.float32)

    pid_src = (src // NP) * NPP + (src % NP)      # padded id of src
    core = dst // NP
    tloc = (dst // NP) * 0 + (dst % NP)           # local dst 0..NP-1
    b_all = pid_src // 128                        # src block 0..NB-1
    t_all = tloc // 128                           # dst tile 0..NT-1
    srclo_all = pid_src % 128
    dstlo_all = tloc % 128

    # bucket ranks per (core, b, t)
    key = (core * NB + b_all) * NT + t_all
    order = np.argsort(key, kind="stable")
    ks = key[order]
    # rank within equal keys
    first = np.ones(len(ks), dtype=bool)
    first[1:] = ks[1:] != ks[:-1]
    starts = np.flatnonzero(first)
    run_id = np.cumsum(first) - 1
    r_sorted = np.arange(len(ks)) - starts[run_id]
    rmax = int(r_sorted.max()) + 1 if len(ks) else 1
    R = max(16, rmax)                             # bucket capacity
    SPB = ((NT * R + 127) // 128) * 128           # slots per block region
    CPB = SPB // 128                              # phase-1 chunks per block
    CPT = (NBM * R) // 128                        # phase-2 chunks per tile

    # srcloc: row vector of src-lo per slot, j = c*128 + p for slot at
    # physical row m = p*CPB + c (transposed one-hot built on device via
    # rank-1 PE broadcast + tensor_tensor is_equal)
    srcloc = np.full((P_CORES, 1, NBM * SPB), PAD, dtype=np.float32)
    dstloc = np.full((P_CORES, 128, NT * CPT), PAD, dtype=np.float32)

    co = core[order]
    bo = b_all[order]
    to = t_all[order]
    so = srclo_all[order]
    do = dstlo_all[order]
    # phase 1: block-region row m = t*R + r at (p=m//CPB, c=m%CPB)
    m = to * R + r_sorted
    p1 = m // CPB
    c1 = m % CPB
    srcloc[co, 0, bo * SPB + c1 * 128 + p1] = so.astype(np.float32)
    # phase 2: tile stream position q = b*R + r at (p=q//CPT, j=q%CPT)
    q = bo * R + r_sorted
    p2 = q // CPT
    j2 = q % CPT
    dstloc[co, p2, to * CPT + j2] = do.astype(np.float32)

    dis_t = np.zeros((P_CORES, 128, NT), dtype=np.float32)
    dinv = np.zeros((P_CORES, 1, NPP), dtype=np.float32)
    for c in range(P_CORES):
        d = dis[c * NP:(c + 1) * NP]
        pad = np.zeros(NPP, dtype=np.float32)
        pad[:NP] = d
        dis_t[c] = pad.reshape(NT, 128).T
        ipad = np.zeros(NPP, dtype=np.float32)
        ipad[:NP] = 1.0 / d
        dinv[c, 0] = ipad

    return dict(dis=dis, R=R, SPB=SPB, CPB=CPB, CPT=CPT,
                srcloc=srcloc, dstloc=dstloc, dis_t=dis_t, dinv=dinv)


# ---------------------------------------------------------------------------
# Bass kernel
# ---------------------------------------------------------------------------

def _build_nc(prep):
    import concourse.bass as bass
    import concourse.bacc as bacc
    import concourse.tile as tile
    from concourse import mybir

    f32 = mybir.dt.float32
    AF = mybir.ActivationFunctionType
    ALU = mybir.AluOpType
    ds = bass.ds

    R, CPB, CPT, SPB = prep["R"], prep["CPB"], prep["CPT"], prep["SPB"]

    nc = bacc.Bacc(
        "TRN2", target_bir_lowering=False, debug=False,
        enable_asserts=False, num_devices=P_CORES,
    )

    H2 = SPB // 2
    xt_d = nc.dram_tensor("xt", [NPAD, DIN], f32, kind="ExternalInput")
    xown_d = nc.dram_tensor("xown", [NPP, DIN], f32, kind="ExternalInput")
    srcloc_d = nc.dram_tensor("srcloc", [1, NBM * SPB], f32,
                              kind="ExternalInput")
    ones1_d = nc.dram_tensor("ones1", [1, 128], f32, kind="ExternalInput")
    iotat_d = nc.dram_tensor("iotat", [128, H2], f32, kind="ExternalInput")
    dstloc_d = nc.dram_tensor("dstloc", [128, NT * CPT], f32,
                              kind="ExternalInput")
    dis_d = nc.dram_tensor("dis_t", [128, NT], f32, kind="ExternalInput")
    dinv_d = nc.dram_tensor("dinv", [1, NPP], f32, kind="ExternalInput")
    w1_d = nc.dram_tensor("w1", [DIN, DH], f32, kind="ExternalInput")
    w2_d = nc.dram_tensor("w2", [DH, DOUT], f32, kind="ExternalInput")
    b1_d = nc.dram_tensor("b1", [DH, 1], f32, kind="ExternalInput")
    b2_d = nc.dram_tensor("b2r", [1, DOUT], f32, kind="ExternalInput")
    iota_d = nc.dram_tensor("iota", [128, 128], f32, kind="ExternalInput")
    ident_d = nc.dram_tensor("ident", [128, 128], f32, kind="ExternalInput")
    out_d = nc.dram_tensor("outp", [NPP, DOUT], f32, kind="ExternalOutput")

    with tile.TileContext(nc) as tc:
        with (
            tc.tile_pool(name="const", bufs=1) as constp,
            tc.tile_pool(name="xb", bufs=3) as xpool,
            tc.tile_pool(name="stg", bufs=3) as stpool,
            tc.tile_pool(name="sl", bufs=3) as slpool,
            tc.tile_pool(name="sp", bufs=4) as spool,
            tc.tile_pool(name="msb", bufs=2) as mpool,
            tc.tile_pool(name="wk", bufs=2) as work,
            tc.tile_pool(name="p1", bufs=2, space="PSUM") as p1pool,
            tc.tile_pool(name="psel", bufs=1, space="PSUM") as pselp,
            tc.tile_pool(name="pagg", bufs=2, space="PSUM") as pagg,
            tc.tile_pool(name="ptr", bufs=1, space="PSUM") as ptr,
            tc.tile_pool(name="pg1", bufs=1, space="PSUM") as pg1,
            tc.tile_pool(name="pg2", bufs=1, space="PSUM") as pg2,
            tc.tile_pool(name="dram", bufs=1, space="DRAM") as dram,
        ):
            iota_sb = constp.tile([128, 128], f32)
            ident_sb = constp.tile([128, 128], f32)
            ones1_sb = constp.tile([1, 128], f32)
            iotat_sb = constp.tile([128, H2], f32)
            w1_sb = constp.tile([DIN, DH], f32)
            w2_sb = constp.tile([DH, DOUT], f32)
            b1_sb = constp.tile([DH, 1], f32)
            b2_sb = constp.tile([1, DOUT], f32)
            for sb, dr in [(iota_sb, iota_d), (ident_sb, ident_d),
                           (ones1_sb, ones1_d), (iotat_sb, iotat_d),
                           (w1_sb, w1_d), (w2_sb, w2_d),
                           (b1_sb, b1_d), (b2_sb, b2_d)]:
                nc.sync.dma_start(sb[:], dr[:])

            # M scratch lives across repeats; zero the pad-block regions
            # once (phase 2 reads them; dstloc=PAD keeps them out of sums,
            # but they must be finite).
            M_d = dram.tile([NBM, SPB * DIN], f32, tag="M", bufs=1)
            zt = work.tile([128, SPB * DIN // 128], f32, tag="zt")
            nc.gpsimd.memset(zt[:], 0.0)
            for b in range(NB, NBM):
                nc.sync.dma_start(M_d[b:b + 1, :], zt[:])

            for _rep in range(int(os.environ.get("GCN_REPEAT", "1"))):
                part = dram.tile([NPP, DOUT], f32, tag="part", bufs=2)
                table = dram.tile([NPAD, DOUT], f32, addr_space="Shared",
                                  tag="table", bufs=2)

                for lidx in range(2):
                    src_d = xt_d if lidx == 0 else table
                    own_d = xown_d if lidx == 0 else part
                    # ---- phase 1: route src blocks into bucket slots ----
                    with tc.For_i(0, NB) as i:
                        xb = xpool.tile([128, DIN], f32, tag="xb")
                        nc.sync.dma_start(xb[:], src_d[ds(i * 128, 128), :])
                        slst = slpool.tile([1, SPB], f32, tag="slst")
                        nc.sync.dma_start(
                            slst[:], srcloc_d[:, ds(i * SPB, SPB)])
                        # transposed one-hot: sel[s, j] = (s == srclo(slot j))
                        sel = spool.tile([128, SPB], f32, tag="sel")
                        for h in range(2):
                            pr = pselp.tile([128, H2], f32, tag="pr")
                            nc.tensor.matmul(
                                pr[:], ones1_sb[:],
                                slst[:, h * H2:(h + 1) * H2],
                                start=True, stop=True)
                            nc.vector.tensor_tensor(
                                sel[:, h * H2:(h + 1) * H2], iotat_sb[:],
                                pr[:], op=ALU.is_equal)
                        ps = p1pool.tile([128, CPB * DIN], f32, tag="ps")
                        for c in range(CPB):
                            nc.tensor.matmul(
                                ps[:, c * DIN:(c + 1) * DIN],
                                sel[:, c * 128:(c + 1) * 128], xb[:],
                                start=True, stop=True)
                        stage = stpool.tile([128, CPB * DIN], f32,
                                            tag="stage")
                        nc.scalar.activation(stage[:], ps[:], AF.Copy,
                                             bias=0.0)
                        nc.sync.dma_start(M_d[ds(i, 1), :], stage[:])
                    # ---- phase 2: per dst tile aggregate + layer tail ----
                    with tc.For_i(0, NT) as i:
                        msb = mpool.tile([128, CPT * DIN], f32, tag="msb")
                        nc.sync.dma_start(
                            msb[:],
                            M_d[:, ds(i * (R * DIN), R * DIN)].rearrange(
                                "b (r f) -> b r f", f=DIN))
                        xdg = xpool.tile([128, DIN], f32, tag="xdg")
                        nc.sync.dma_start(xdg[:], own_d[ds(i * 128, 128), :])
                        dlst = slpool.tile([128, CPT], f32, tag="dlst")
                        nc.sync.dma_start(
                            dlst[:], dstloc_d[:, ds(i * CPT, CPT)])
                        dcol = slpool.tile([128, 1], f32, tag="dcol")
                        nc.sync.dma_start(dcol[:], dis_d[:, ds(i, 1)])
                        agg = pagg.tile([128, DIN], f32, tag="agg")
                        for j in range(CPT):
                            s_t = spool.tile([128, 128], f32, tag="s2")
                            nc.vector.tensor_scalar(
                                s_t[:], iota_sb[:], dlst[:, j:j + 1], None,
                                ALU.is_equal)
                            nc.tensor.matmul(
                                agg[:], s_t[:],
                                msb[:, j * DIN:(j + 1) * DIN],
                                start=(j == 0), stop=False)
                        nc.tensor.matmul(agg[:], ident_sb[:], xdg[:],
                                         start=False, stop=(lidx == 0))
                        if lidx == 0:
                            a_sb = work.tile([128, DIN], f32, tag="a_sb")
                            nc.scalar.activation(a_sb[:], agg[:], AF.Copy,
                                                 bias=0.0,
                                                 scale=dcol[:, 0:1])
                            aT_p = ptr.tile([DIN, 128], f32, tag="aT")
                            nc.tensor.transpose(aT_p[:], a_sb[:], ident_sb[:])
                            aT_sb = work.tile([DIN, 128], f32, tag="aT_sb")
                            nc.scalar.activation(aT_sb[:], aT_p[:], AF.Copy,
                                                 bias=0.0)
                            x1_p = pg1.tile([DH, 128], f32, tag="x1")
                            nc.tensor.matmul(x1_p[:], w1_sb[:], aT_sb[:],
                                             start=True, stop=True)
                            x1_sb = work.tile([DH, 128], f32, tag="x1_sb")
                            nc.scalar.activation(x1_sb[:], x1_p[:], AF.Lrelu,
                                                 bias=b1_sb[:, 0:1],
                                                 alpha=NEG_SLOPE)
                            x2_p = pg2.tile([128, DOUT], f32, tag="x2")
                            nc.tensor.matmul(x2_p[:], x1_sb[:], w2_sb[:],
                                             start=True, stop=True)
                            x2_sb = work.tile([128, DOUT], f32, tag="x2_sb")
                            nc.scalar.activation(x2_sb[:], x2_p[:], AF.Copy,
                                                 bias=0.0,
                                                 scale=dcol[:, 0:1])
                            nc.sync.dma_start(
                                part[ds(i * 128, 128), :], x2_sb[:])
                        else:
                            div = slpool.tile([1, 128], f32, tag="div")
                            nc.sync.dma_start(
                                div[:], dinv_d[:, ds(i * 128, 128)])
                            nc.tensor.matmul(agg[:], div[:], b2_sb[:],
                                             start=False, stop=True)
                            o_sb = work.tile([128, DOUT], f32, tag="o_sb")
                            nc.scalar.activation(o_sb[:], agg[:], AF.Lrelu,
                                                 bias=0.0,
                                                 scale=dcol[:, 0:1],
                                                 alpha=NEG_SLOPE)
                            nc.sync.dma_start(
                                out_d[ds(i * 128, 128), :], o_sb[:])
                    if lidx == 0:
                        nc.gpsimd.collective_compute(
                            "AllGather", mybir.AluOpType.bypass,
                            replica_groups=[list(range(P_CORES))],
                            ins=[part.opt()], outs=[table.opt()],
                        )

    nc.compile()
    return nc


def _make_in_maps(inputs, W1, b1, W2, b2, prep):
    dis = prep["dis"]
    xt = np.zeros((NPAD, DIN), dtype=np.float32)
    x32 = np.asarray(inputs, np.float32)
    for c in range(P_CORES):
        xt[c * NPP:c * NPP + NP] = (x32[c * NP:(c + 1) * NP]
                                    * dis[c * NP:(c + 1) * NP, None])
    iota = np.tile(np.arange(128, dtype=np.float32), (128, 1))
    ident = np.eye(128, dtype=np.float32)
    in_maps = []
    for c in range(P_CORES):
        in_maps.append({
            "xt": xt,
            "xown": xt[c * NPP:(c + 1) * NPP],
            "srcloc": prep["srcloc"][c],
            "dstloc": prep["dstloc"][c],
            "dis_t": prep["dis_t"][c],
            "dinv": prep["dinv"][c],
            "w1": np.asarray(W1, np.float32),
            "w2": np.asarray(W2, np.float32),
            "b1": np.asarray(b1, np.float32).reshape(DH, 1),
            "b2r": np.asarray(b2, np.float32).reshape(1, DOUT),
            "iota": iota,
            "ident": ident,
            "ones1": np.ones((1, 128), dtype=np.float32),
            "iotat": np.tile(
                np.arange(128, dtype=np.float32)[:, None],
                (1, prep["SPB"] // 2)),
        })
    return in_maps


_CACHE = {}


def kernel(inputs, edge_index, W1, b1, W2, b2, _trace=False, _results_box=None):
    from concourse.bass_utils import run_bass_kernel_spmd

    edge_index = np.asarray(edge_index)
    key = hashlib.sha1(edge_index.tobytes()).hexdigest()
    key += ":r%s" % os.environ.get("GCN_REPEAT", "1")
    if key not in _CACHE:
        prep = _prep(edge_index)
        nc = _build_nc(prep)
        _CACHE[key] = (prep, nc)
    prep, nc = _CACHE[key]
    in_maps = _make_in_maps(inputs, W1, b1, W2, b2, prep)
    res = run_bass_kernel_spmd(
        nc, in_maps, core_ids=list(range(P_CORES)), trace=_trace,
    )
    if _results_box is not None:
        _results_box.append(res)
    out = np.concatenate(
        [res.results[c]["outp"][:NP] for c in range(P_CORES)], axis=0
    )
    return out.astype(np.float32)
